# revision 30
# baseline (speedup 1.0000x reference)
import numpy as np

# nn_MultiHeadedAttention: B=4, S=2048, D_MODEL=1024, H=16, D_K=64, fp32.
# Sharding: 8 cores = 4 batches x 2 head-groups (8 heads each).
# All-bf16 fused pipeline: projections, V, and out-projection interleave
# into the attention unit stream (p-major) so the PE fills the gaps left
# by ACT exp pacing. Softmax denominator rides as a ones column in V;
# normalization = masked-ones PE broadcast of both denominators + one
# wide DVE reciprocal + muls. Host sums the two head-group partials.

B, S, D, H, DK = 4, 2048, 1024, 16, 64
NCORES = 8
DG = 512  # dims per head-group (8 heads x 64)

_NC_CACHE = {}
LAST_EXEC_NS = None


def _build_nc():
    import concourse.bacc as bacc
    import concourse.tile as tile
    from concourse import mybir

    F32 = mybir.dt.float32
    BF16 = mybir.dt.bfloat16
    EXP = mybir.ActivationFunctionType.Exp

    nc = bacc.Bacc(None, target_bir_lowering=False, debug=True)

    xqT = nc.dram_tensor("xqT", [D, S], BF16, kind="ExternalInput")
    xkT = nc.dram_tensor("xkT", [D, S], BF16, kind="ExternalInput")
    xvT = nc.dram_tensor("xvT", [D, S], BF16, kind="ExternalInput")
    wqT = nc.dram_tensor("wqT", [D, DG], BF16, kind="ExternalInput")
    wkT = nc.dram_tensor("wkT", [D, DG], BF16, kind="ExternalInput")
    wvT = nc.dram_tensor("wvT", [D, DG], BF16, kind="ExternalInput")
    woT = nc.dram_tensor("woT", [DG, D], BF16, kind="ExternalInput")
    bqc = nc.dram_tensor("bqc", [128, 4], F32, kind="ExternalInput")
    bkc = nc.dram_tensor("bkc", [128, 4], F32, kind="ExternalInput")
    bvr = nc.dram_tensor("bvr", [DG], F32, kind="ExternalInput")
    y_d = nc.dram_tensor("y", [S, D], BF16, kind="ExternalOutput")

    import concourse.bass as bass

    with (
        tile.TileContext(nc) as tc,
        nc.allow_low_precision(reason="bf16 within rel-err budget"),
        tc.tile_pool(name="persist", bufs=1) as persist,
        tc.tile_pool(name="stage", bufs=2) as stage,
        tc.tile_pool(name="attn_sb", bufs=3) as asb,
        tc.tile_pool(name="ps_st", bufs=2, space="PSUM") as ps_st,
        tc.tile_pool(name="ps_mix", bufs=1, space="PSUM") as ps_mix,
    ):
        QT = [persist.tile([128, S], BF16, name=f"QT{p}") for p in range(4)]
        KT = [persist.tile([128, S], BF16, name=f"KT{p}") for p in range(4)]
        AT = [persist.tile([128, S], BF16, name=f"AT{p}") for p in range(4)]
        VO = [persist.tile([128, 8, 128], BF16, name=f"VO{s}") for s in range(16)]
        wq = persist.tile([128, 8, DG], BF16, name="wq")
        wk = persist.tile([128, 8, DG], BF16, name="wk")
        wv = persist.tile([128, 8, DG], BF16, name="wv")
        wo = persist.tile([128, 4, D], BF16, name="wo")
        bq_sb = persist.tile([128, 4], F32, name="bq_sb")
        bk_sb = persist.tile([128, 4], F32, name="bk_sb")
        bv_sb = persist.tile([128, DG], F32, name="bv_sb")
        # row h of the broadcast stationary selects PE cols h*64..h*64+63
        osel = persist.tile([1, 2, 128], BF16, name="osel")

        nc.gpsimd.dma_start(bq_sb[:], bqc[:])
        nc.gpsimd.dma_start(bk_sb[:], bkc[:])
        bv_ap = bvr[:]
        bv_bcast = bass.AP(tensor=bv_ap.tensor, offset=bv_ap.offset, ap=[[0, 128], *bv_ap.ap])
        nc.gpsimd.dma_start(bv_sb[:], bv_bcast)
        nc.vector.memset(osel[:], 0.0)
        nc.vector.memset(osel[:, 0, 0:64], 1.0)
        nc.vector.memset(osel[:, 1, 64:128], 1.0)
        for s in range(16):
            nc.vector.memset(VO[s][:], 0.0)
            nc.vector.memset(VO[s][:, :, 64:65], 1.0)

        for i in range(8):
            nc.gpsimd.dma_start(wk[:, i, :], wkT[i * 128 : (i + 1) * 128, :])
        for i in range(8):
            nc.gpsimd.dma_start(wq[:, i, :], wqT[i * 128 : (i + 1) * 128, :])
        for i in range(8):
            nc.gpsimd.dma_start(wv[:, i, :], wvT[i * 128 : (i + 1) * 128, :])
        for p in range(4):
            nc.gpsimd.dma_start(wo[:, p, :], woT[p * 128 : (p + 1) * 128, :])

        # ---------- emission helpers ----------
        def load_xs(x_d, qc, eng):
            xs = stage.tile([128, 8, 512], BF16, name="xs", bufs=3)
            qs = slice(qc * 512, (qc + 1) * 512)
            for i in range(8):
                eng.dma_start(xs[:, i, :], x_d[i * 128 : (i + 1) * 128, qs])
            return xs

        def st_half():
            # all non-pv PSUM shares the "st" ring (bank budget: st 4,
            # pv0/pv1 2+2 = 8); callers use the first half of the tile.
            return ps_st.tile([128, 2, 512], F32, name="st")[:, 0, :]

        def qk_group(xs, wt, b_sb, OUT, qc, ps):
            qs = slice(qc * 512, (qc + 1) * 512)
            for p in ps:
                pp = st_half()
                for i in range(8):
                    nc.tensor.matmul(
                        pp,
                        wt[:, i, p * 128 : (p + 1) * 128],
                        xs[:, i, :],
                        start=(i == 0),
                        stop=(i == 7),
                    )
                nc.vector.tensor_scalar_add(OUT[p][:, qs], pp, b_sb[:, p : p + 1])

        def v_pair_mm(xv, pj):
            for half in range(2):
                sb = 2 * pj + half
                vp = st_half()
                for i in range(8):
                    nc.tensor.matmul(
                        vp,
                        xv[:, i, half * 128 : (half + 1) * 128],
                        wv[:, i, :],
                        start=(i == 0),
                        stop=(i == 7),
                    )
                nc.vector.tensor_add(
                    VO[sb][:, :, 0:64],
                    vp.rearrange("p (h d) -> p h d", h=8),
                    bv_sb[:].rearrange("p (h d) -> p h d", h=8),
                )

        def out_sb(sb):
            ss = slice(sb * 128, (sb + 1) * 128)
            ys = asb.tile([128, 2, 512], BF16, name="ys", bufs=3)
            for oc in range(2):
                yp = st_half()
                for p in range(4):
                    nc.tensor.matmul(
                        yp,
                        AT[p][:, ss],
                        wo[:, p, oc * 512 : (oc + 1) * 512],
                        start=(p == 0),
                        stop=(p == 3),
                    )
                nc.vector.tensor_copy(ys[:, oc, :], yp)
            eng = (nc.gpsimd, nc.sync)[sb % 2]
            eng.dma_start(y_d[ss, :], ys[:])

        # ---------- attention unit ----------
        # Per slot: scores(kb) -> PV(kb-2) (exp lags ~2 chunks) -> filler.
        # The last two PVs + normalization are carried into the next unit's
        # first slots so the PE never waits on the trailing exps.
        def unit(p, qc, fillers, carry_in):
            qs = slice(qc * 512, (qc + 1) * 512)
            pv = [ps_mix.tile([128, 512], F32, name=f"pv{h}", bufs=2) for h in range(2)]
            ech = []
            work = list(carry_in)
            li = 0
            ci = 0

            def pump():
                nonlocal li, ci
                while li < len(fillers) and li < ci + 2:
                    fillers[li][0]()
                    li += 1
                if work:
                    work.pop(0)()
                elif ci < len(fillers):
                    fillers[ci][1]()
                    ci += 1

            def sc_kb(kb):
                st = ps_st.tile([128, 2, 512], F32, name="st")
                for h in range(2):
                    nc.tensor.matmul(
                        st[:, h, :],
                        KT[p][h * 64 : (h + 1) * 64, kb * 128 : (kb + 1) * 128],
                        QT[p][h * 64 : (h + 1) * 64, qs],
                        start=True,
                        stop=True,
                    )
                ec = asb.tile([128, 2, 512], BF16, name="ech", bufs=10)
                ech.append(ec)
                nc.scalar.activation(out=ec[:], in_=st[:], func=EXP, scale=0.125)

            def pv_kb(kb):
                for h in range(2):
                    nc.tensor.matmul(
                        pv[h][:],
                        VO[kb][:, p * 2 + h, :],
                        ech[kb][:, h, :],
                        start=(kb == 0),
                        stop=(kb == 15),
                    )

            def norm():
                den2 = asb.tile([1, 2, 512], BF16, name="den2", bufs=2)
                for h in range(2):
                    nc.vector.tensor_copy(den2[:, h, :], pv[h][64:65, :])
                bc = st_half()
                for h in range(2):
                    nc.tensor.matmul(
                        bc, osel[:, h, :], den2[:, h, :], start=(h == 0), stop=(h == 1)
                    )
                bcs = asb.tile([128, 512], F32, name="bcs", bufs=2)
                nc.vector.reciprocal(bcs[:], bc)
                for h in range(2):
                    hb = h * 64
                    nc.vector.tensor_mul(
                        AT[p][hb : hb + 64, qs], pv[h][0:64, :], bcs[hb : hb + 64, :]
                    )

            for kb in range(16):
                sc_kb(kb)
                if kb >= 2:
                    pv_kb(kb - 2)
                pump()
            while work or ci < len(fillers):
                pump()
            return [F(pv_kb, 14), F(pv_kb, 15), norm]

        # ---------- prologue: K qc*, Q qc0 for head-pair block p=0 ----------
        xk = {}
        xq = {}
        xk[0] = load_xs(xkT, 0, nc.sync)
        qk_group(xk[0], wk, bk_sb, KT, 0, [0])
        xq[0] = load_xs(xqT, 0, nc.scalar)
        qk_group(xq[0], wq, bq_sb, QT, 0, [0])
        for qc in range(1, 4):
            xk[qc] = load_xs(xkT, qc, nc.sync)
            qk_group(xk[qc], wk, bk_sb, KT, qc, [0])

        # ---------- filler schedules per unit ----------
        def F(fn, *a):
            return lambda: fn(*a)

        def k_rest(qc):
            box = {}
            return (
                lambda: box.setdefault("xs", load_xs(xkT, qc, nc.sync)),
                lambda: qk_group(box["xs"], wk, bk_sb, KT, qc, [1, 2, 3]),
            )

        def q_blk(qc, ps):
            box = {}
            return (
                lambda: box.setdefault("xs", load_xs(xqT, qc, nc.gpsimd)),
                lambda: qk_group(box["xs"], wq, bq_sb, QT, qc, ps),
            )

        def v_pair_split(pj):
            box = {}

            def load():
                ss = slice(pj * 256, (pj + 1) * 256)
                xv = stage.tile([128, 8, 256], BF16, name="xv", bufs=4)
                for i in range(8):
                    nc.sync.dma_start(xv[:, i, :], xvT[i * 128 : (i + 1) * 128, ss])
                box["xv"] = xv

            return (load, lambda: v_pair_mm(box["xv"], pj))

        def as_pair(comp):
            return (lambda: None, comp)

        # QT/KT block p must be fully projected before unit (p, *) starts:
        # KT p1-3 and QT p1 land during p=0's units, QT p2-3 during p=1's.
        fill = {
            (0, 0): [v_pair_split(j) for j in range(8)] + [q_blk(1, [0])],
            (0, 1): [k_rest(0), q_blk(2, [0]), q_blk(0, [1])],
            (0, 2): [k_rest(1), q_blk(3, [0]), q_blk(1, [1])],
            (0, 3): [k_rest(2), k_rest(3), q_blk(2, [1]), q_blk(3, [1])],
            (1, 0): [q_blk(0, [2, 3])],
            (1, 1): [q_blk(1, [2, 3])],
            (1, 2): [q_blk(2, [2, 3])],
            (1, 3): [q_blk(3, [2, 3])],
            (3, 1): [as_pair(F(out_sb, sb)) for sb in range(0, 4)],
            (3, 2): [as_pair(F(out_sb, sb)) for sb in range(4, 8)],
            (3, 3): [as_pair(F(out_sb, sb)) for sb in range(8, 12)],
        }

        carry = []
        for p in range(4):
            for qc in range(4):
                carry = unit(p, qc, fill.get((p, qc), []), carry)
        for w in carry:
            w()
        for sb in range(12, 16):
            out_sb(sb)

    nc.compile()
    return nc


def _get_nc():
    if "nc" not in _NC_CACHE:
        _NC_CACHE["nc"] = _build_nc()
    return _NC_CACHE["nc"]


def kernel(**inputs):
    import ml_dtypes
    from concourse import bass_utils

    BF = ml_dtypes.bfloat16
    q, k, v = inputs["query"], inputs["key"], inputs["value"]
    Wq, Wk, Wv, Wo = inputs["Wq"], inputs["Wk"], inputs["Wv"], inputs["Wo"]
    bq, bk, bv, bo = inputs["bq"], inputs["bk"], inputs["bv"], inputs["bo"]

    nc = _get_nc()
    in_maps = []
    for c in range(NCORES):
        b, hg = divmod(c, 2)
        r0 = hg * DG
        rs = slice(r0, r0 + DG)
        in_maps.append(
            {
                "xqT": np.ascontiguousarray(q[b].T).astype(BF),
                "xkT": np.ascontiguousarray(k[b].T).astype(BF),
                "xvT": np.ascontiguousarray(v[b].T).astype(BF),
                "wqT": np.ascontiguousarray(Wq[rs, :].T).astype(BF),
                "wkT": np.ascontiguousarray(Wk[rs, :].T).astype(BF),
                "wvT": np.ascontiguousarray(Wv[rs, :].T).astype(BF),
                "woT": np.ascontiguousarray(Wo[:, rs].T).astype(BF),
                "bqc": np.ascontiguousarray(bq[rs].reshape(4, 128).T),
                "bkc": np.ascontiguousarray(bk[rs].reshape(4, 128).T),
                "bvr": np.ascontiguousarray(bv[rs]),
            }
        )
    import os

    trace = bool(os.environ.get("KERNEL_TRACE"))
    res = bass_utils.run_bass_kernel_spmd(
        nc, in_maps, core_ids=list(range(NCORES)), trace=trace
    )
    global LAST_EXEC_NS
    LAST_EXEC_NS = res.exec_time_ns
    out = np.empty((B, S, D), np.float32)
    for b in range(B):
        out[b] = (
            res.results[2 * b]["y"].astype(np.float32)
            + res.results[2 * b + 1]["y"].astype(np.float32)
            + bo[None, :]
        )
    return out
